# revision 1
# baseline (speedup 1.0000x reference)
"""Trainium2 Bass kernel for ChebConv with spatial attention.

Reference computation (per sample b):
    A_k = cheb[k] * att[b]                    (elementwise, [N,N])
    rhs_k = A_k @ x[b,t]                      ([N,N] @ [N,F_IN] for all t)
    out[b,t] = relu(sum_k rhs_k @ Theta[k])   ([N,F_OUT])

Sharding: data-parallel over batch B=8, one sample per NeuronCore.
cheb/Theta replicated. Host prep passes transposed adjacency factors
(attT/chebT, layout [j,i], cast to bf16) so the on-chip elementwise
product directly yields A^T tiles, which the PE consumes as the moving
matmul operand with contraction over j on the partition dim — no
on-chip transposes. All matmuls run in bf16 (single-pass on the PE,
fp32 PSUM accumulation); the relu'd output is stored bf16 on-chip and
upcast to fp32 on the host.

Per-core dataflow (phase B k=0 is DMA-paced near the 358 GB/s HBM
line rate, so the schedule is built around keeping both HWDGE queues
streaming and hiding the ~1.3us DMA completion-receipt latency):
  phase B: per (k, j-tile): AT = att_u8*chebT (DVE, bf16 out), then 8
           matmuls RT[(t,f)=128, i=512] += X_tile^T @ AT into 8 PSUM
           chains (one per tg/ih), j-accumulated; after each k,
           staggered PSUM->SBUF bf16 casts (DVE even / ACT odd chains).
           The next k's j0/j1 cheb tiles + products are prefetched
           mid-k (sync queue is empty of this k's traffic by group 3,
           DVE has idle slack) so the k-boundary only waits on the
           casts (~0.4us).
  phase C: out[i=128, (t,o)] += RT^T @ thetaM_k per tg in order
           0,1(psA) 2,3(psB); thetaM zero-pads Theta[k] per 32-row
           strip so one N=256 matmul covers a whole t-group. relu(psA)
           on ACT (store on sync) and relu(psB) on DVE (store on
           scalar); the final block's last relu/store is quarter-sized
           to shorten the end-of-kernel DMA receipt tail. Phase C PSUM
           tiles reuse the chain{c} pool tags so banks free per-chain.

DMA layout: att (uint8, 1/255 folded into chebT on the host), x and
cheb stream as grouped row-block tiles (two narrow j0/j1 groups for a
fast start, then 2-j-tile 0.5 MB groups to stay above the ~1.4/us
per-queue HWDGE issue rate), alternating between the sync and scalar
queues so each carries ~half the k0-critical bytes; the very first
att/cheb transfers are half-tiles split across both queues. theta
loads after the k0 stream. ~4us of N=128 warm-up matmuls on a
dedicated zero tile run during the DMA head so the PE HAM clock gate
releases (1.2 -> 2.4 GHz) right as the first real matmul is ready.

Measured (best of 3, device-noise +-2-4us from the P0 clock state):
~75.2us vs 80.4us baseline. Floor accounting: 51.2us PE streaming
(98304 + 24576 cycles @ 2.4 GHz) + ~6us DMA head + ~2us k0 receipt
jitter + ~12us fixed TileContext drain/butterfly/sem-clear tail.
"""

import numpy as np
from contextlib import ExitStack

B, T, N, F_IN, F_OUT, K = 8, 16, 1024, 32, 64, 3
NJ = N // 128  # j tiles (contraction)
NI = N // 128  # i tiles (output rows)
NTG = 4        # t-groups of 4 t's -> 128 = 4*32 partitions
TF = T * F_IN   # 512
TO = T * F_OUT  # 1024
WARMUP = 44

_LAST_RESULTS = None  # BassKernelResults of the most recent run (for test harness)


def _build_bass():
    import concourse.mybir as mybir
    import concourse.tile as tile
    from concourse import bacc
    from concourse.bass import ts

    f32 = mybir.dt.float32
    bf16 = mybir.dt.bfloat16
    u8 = mybir.dt.uint8
    nc = bacc.Bacc()

    xT_d = nc.dram_tensor("xT", [N, TF], bf16, kind="ExternalInput")
    attT_d = nc.dram_tensor("attT", [N, N], u8, kind="ExternalInput")
    chebT_d = nc.dram_tensor("chebT", [K * N, N], bf16, kind="ExternalInput")
    th_d = nc.dram_tensor("thetaM", [128, K * 4 * F_OUT], bf16, kind="ExternalInput")
    out_d = nc.dram_tensor("out", [N, TO], bf16, kind="ExternalOutput")

    with tile.TileContext(nc) as tc, ExitStack() as ctx:
        x_pool = ctx.enter_context(tc.tile_pool(name="x", bufs=1))
        att_pool = ctx.enter_context(tc.tile_pool(name="att", bufs=1))
        cheb_pool = ctx.enter_context(tc.tile_pool(name="cheb", bufs=4))
        at_pool = ctx.enter_context(tc.tile_pool(name="at", bufs=6))
        rt_pool = ctx.enter_context(tc.tile_pool(name="rt", bufs=K * NTG))
        th_pool = ctx.enter_context(tc.tile_pool(name="th", bufs=1))
        ob_pool = ctx.enter_context(tc.tile_pool(name="ob", bufs=3))
        wz_pool = ctx.enter_context(tc.tile_pool(name="wz", bufs=1))

        # queue alternation: sync and scalar HWDGE rings carry ~equal bytes
        q = [nc.sync, nc.scalar]

        # j-tiles grouped per DMA: two narrow leading groups let the first
        # matmul start early; the rest go wide so the HWDGE issue rate
        # (~1.4 issues/us/queue) doesn't starve the stream
        GROUPS = [(0,), (1,), (2, 3), (4, 5), (6, 7)]

        def grouped(dram, row0, L):  # L j-tiles -> [128, L, cols]
            return dram[row0:row0 + L * 128, :].rearrange("(a p) n -> p a n", p=128)

        def g3(t, L, cols):  # view a grouped SBUF tile as [128, L, cols]
            return t[:].rearrange("p (a n) -> p a n", a=L)

        xg, attg = [None] * len(GROUPS), [None] * len(GROUPS)

        # phase B: RT[k][tg] = X[:, tg-block]^T @ (attT * chebT_k)
        rts = [[None] * NTG for _ in range(K)]
        th = None
        with tc.tile_pool(name="psumB", bufs=1, space="PSUM") as pb:
            # PE warm-up on a dedicated zero tile: short N=128 matmuls so
            # HAM un-throttles right as the first real matmul data lands.
            # Shares the last chain's PSUM bank; drained before phase B
            # reaches it.
            wz = wz_pool.tile([128, 128], bf16, name="warmz")
            nc.gpsimd.memset(wz[:], 0)
            wps = pb.tile([128, 512], f32, name="warmps", tag="chain7")
            for _ in range(WARMUP):
                nc.tensor.matmul(wps[:, 0:128], wz[:], wz[:], start=True, stop=True)
            at_pre = {}
            for k in range(K):
                chains = [
                    pb.tile([128, 512], f32, name=f"chain{k}_{c}", tag=f"chain{c}")
                    for c in range(2 * NTG)
                ]
                for g, grp in enumerate(GROUPS):
                    L = len(grp)
                    row0 = grp[0] * 128
                    first = k == 0 and g == 0
                    pre = k > 0 and g < 2
                    if k == 0:
                        a = att_pool.tile([128, L * N], u8,
                                          name=f"attg{g}", tag=f"attg{g}")
                        xt = x_pool.tile([128, L * TF], bf16,
                                         name=f"xg{g}", tag=f"xg{g}")
                        if first:
                            # halve the very first transfers across both
                            # queues so the i-half 0 product (all the ih=0
                            # matmuls need) is ready ~1us earlier; h1 and x
                            # are emitted after cheb-h0 below
                            nc.sync.dma_start(a[:, 0:512], attT_d[0:128, 0:512])
                        else:
                            q[g % 2].dma_start(g3(a, L, N),
                                               grouped(attT_d, row0, L))
                        attg[g] = a
                        xg[g] = xt
                    if not pre:
                        cb = cheb_pool.tile([128, L * N], bf16, name=f"cb{k}_{g}",
                                            tag="cb", padded_shape=[128, 2 * N])
                        if first:
                            nc.scalar.dma_start(cb[:, 0:512],
                                                chebT_d[0:128, 0:512])
                            nc.sync.dma_start(cb[:, 512:1024],
                                              chebT_d[0:128, 512:1024])
                            nc.scalar.dma_start(a[:, 512:1024],
                                                attT_d[0:128, 512:1024])
                            nc.scalar.dma_start(xt[:], xT_d[0:128, :])
                        else:
                            q[(g + 1) % 2].dma_start(
                                g3(cb, L, N), grouped(chebT_d, k * N + row0, L))
                    if k == 0 and not first:
                        # x rides behind att+cheb: the mult needs att/cheb
                        # first, the matmuls only need x ~0.7us later
                        q[g % 2].dma_start(g3(xt, L, TF),
                                           grouped(xT_d, row0, L))
                    for js in range(L):
                        j = grp[js]
                        if pre:
                            at = at_pre.pop((k, j))
                        else:
                            at = at_pool.tile([128, N], bf16, name=f"at{k}_{j}",
                                              tag="at")
                            if first:
                                nc.vector.tensor_mul(at[:, 0:512], a[:, 0:512],
                                                     cb[:, 0:512])
                                nc.vector.tensor_mul(at[:, 512:1024],
                                                     a[:, 512:1024],
                                                     cb[:, 512:1024])
                            else:
                                nc.vector.tensor_mul(
                                    at[:], attg[g][:, ts(js, N)],
                                    cb[:, ts(js, N)])
                        for ih in range(2):
                            for tg in range(NTG):
                                nc.tensor.matmul(
                                    chains[tg * 2 + ih][:],
                                    xg[g][:, js * TF + tg * 128:
                                          js * TF + (tg + 1) * 128],
                                    at[:, ts(ih, 512)],
                                    start=(j == 0),
                                    stop=(j == NJ - 1),
                                )
                    if g == 3 and k < K - 1:
                        # prefetch next k's j0/j1 products mid-k (sync queue
                        # has nothing of this k left, DVE has idle slack) so
                        # the k-boundary only has the chain casts left to do
                        kn = k + 1
                        cbp = cheb_pool.tile([128, 2 * N], bf16,
                                             name=f"cbp{kn}", tag="cb")
                        nc.sync.dma_start(g3(cbp, 2, N),
                                          grouped(chebT_d, kn * N, 2))
                        for jp in range(2):
                            atp = at_pool.tile([128, N], bf16,
                                               name=f"atp{kn}_{jp}", tag="at")
                            nc.vector.tensor_mul(
                                atp[:], attg[jp][:, 0:N], cbp[:, ts(jp, N)])
                            at_pre[(kn, jp)] = atp
                if k == 0:
                    # theta only needed by phase C; keep it off the
                    # k0-critical DMA window
                    th = th_pool.tile([128, K * 4 * F_OUT], bf16)
                    nc.scalar.dma_start(th[:], th_d[:, :])
                for tg in range(NTG):
                    rt = rt_pool.tile([128, N], bf16)
                    nc.vector.tensor_copy(rt[:, 0:512], chains[tg * 2][:])
                    nc.scalar.copy(rt[:, 512:1024], chains[tg * 2 + 1][:])
                    rts[k][tg] = rt

            # phase C: out[i-block, (t,o)] = relu(sum_k RT_k^T @ thetaM_k).
            # One matmul per (tg, k): full K=128 contraction where thetaM
            # zero-pads Theta[k] per 32-row strip, producing the 4 t's of
            # the t-group in one N=256 matmul. tg order 0,1 (psA) then
            # 2,3 (psB): psA is complete at 50% of the i-block so its
            # relu (ACT) and half-block store overlap the psB matmuls;
            # relu(psB) runs on DVE. PSUM tiles reuse the chain{c} tags
            # so each bank frees as soon as its k=2 chain is cast, not
            # at pool teardown. Stores ride the sync queue (idle here).
            for ib in range(NI):
                psA = pb.tile([128, 512], f32, name=f"psA{ib}",
                              tag=f"chain{(2 * ib) % 8}")
                psB = pb.tile([128, 512], f32, name=f"psB{ib}",
                              tag=f"chain{(2 * ib + 1) % 8}")
                for tg, ps in ((0, psA), (1, psA), (2, psB), (3, psB)):
                    for k in range(K):
                        nc.tensor.matmul(
                            ps[:, ts(tg % 2, 4 * F_OUT)],
                            rts[k][tg][:, ts(ib, 128)],
                            th[:, ts(k, 4 * F_OUT)],
                            start=(k == 0),
                            stop=(k == K - 1),
                        )
                ob = ob_pool.tile([128, TO], bf16)
                nc.scalar.activation(ob[:, 0:512], psA[:],
                                     mybir.ActivationFunctionType.Relu)
                nc.sync.dma_start(out_d[ts(ib, 128), 0:512], ob[:, 0:512])
                if ib < NI - 1:
                    nc.vector.tensor_scalar_max(ob[:, 512:1024], psB[:], 0.0)
                    nc.scalar.dma_start(out_d[ts(ib, 128), 512:1024],
                                        ob[:, 512:1024])
                else:
                    # quarter-split the final block so the very last store is
                    # small and issues right after a short relu
                    nc.vector.tensor_scalar_max(ob[:, 512:768], psB[:, 0:256],
                                                0.0)
                    nc.scalar.dma_start(out_d[ts(ib, 128), 512:768],
                                        ob[:, 512:768])
                    nc.vector.tensor_scalar_max(ob[:, 768:1024], psB[:, 256:512],
                                                0.0)
                    nc.sync.dma_start(out_d[ts(ib, 128), 768:1024],
                                      ob[:, 768:1024])

    nc.compile()
    return nc


def _prep_inputs(x, att, cheb, Theta):
    import ml_dtypes

    bf16 = ml_dtypes.bfloat16
    # att rides as uint8 (att ~ U[0,1) so fixed-point abs err <= 2e-3);
    # the 1/255 rescale folds into chebT so the on-chip product
    # att_u8 * chebT_scaled == att * chebT.
    chebT = np.ascontiguousarray(cheb.transpose(0, 2, 1)).reshape(K * N, N)
    chebT = (chebT * (1.0 / 255.0)).astype(bf16)
    # zero-padded Theta: strip tt of the partition dim carries Theta[k]
    # only in the tt-th 64-col block of k's 256-col group
    thetaM = np.zeros((128, K * 4 * F_OUT), np.float32)
    for tt in range(4):
        for k in range(K):
            thetaM[tt * 32:(tt + 1) * 32,
                   k * 4 * F_OUT + tt * F_OUT:
                   k * 4 * F_OUT + (tt + 1) * F_OUT] = Theta[k]
    thetaM = thetaM.astype(bf16)

    in_maps = []
    for b in range(B):
        in_maps.append({
            "xT": np.ascontiguousarray(
                x[b].transpose(1, 0, 2)).reshape(N, TF).astype(bf16),
            "attT": np.rint(np.ascontiguousarray(att[b].T) * 255.0
                            ).astype(np.uint8),
            "chebT": chebT,
            "thetaM": thetaM,
        })
    return in_maps


def kernel(**inputs: np.ndarray) -> np.ndarray:
    global _LAST_RESULTS
    from concourse.bass_utils import run_bass_kernel_spmd

    x = np.asarray(inputs["x"], dtype=np.float32)
    att = np.asarray(inputs["spatial_attention"], dtype=np.float32)
    cheb = np.asarray(inputs["cheb"], dtype=np.float32)
    Theta = np.asarray(inputs["Theta"], dtype=np.float32)

    in_maps = _prep_inputs(x, att, cheb, Theta)
    nc = _build_bass()
    res = run_bass_kernel_spmd(nc, in_maps, core_ids=list(range(B)))
    _LAST_RESULTS = res

    out = np.stack(
        [r["out"].astype(np.float32).reshape(N, T, F_OUT).transpose(1, 0, 2)
         for r in res.results]
    )
    return out



# revision 4
# speedup vs baseline: 1.0256x; 1.0256x over previous
"""Trainium2 Bass kernel for ChebConv with spatial attention.

Reference computation (per sample b):
    A_k = cheb[k] * att[b]                    (elementwise, [N,N])
    rhs_k = A_k @ x[b,t]                      ([N,N] @ [N,F_IN] for all t)
    out[b,t] = relu(sum_k rhs_k @ Theta[k])   ([N,F_OUT])

Sharding: data-parallel over batch B=8, one sample per NeuronCore.
cheb/Theta replicated. Host prep passes transposed adjacency factors
(attT uint8, chebT int8) so the on-chip elementwise product directly
yields A^T tiles, which the PE consumes as the moving matmul operand
with contraction over j on the partition dim — no on-chip transposes.

Quantization: att rides as uint8 (U[0,1) fixed point); cheb rides as
int8 with PER-J scales s[j] = max_{k,i}|cheb[k,i,j]|/127; both the
1/255 and s[j] rescales fold into xT rows on the host (RT contracts
over j, so a per-j factor on x is exact). Halving cheb (the 6 MB
dominant stream) drops the k0 DMA requirement from ~300 GB/s to
~220 GB/s so phase B never stalls on receipts. Measured rel err
~9e-3 vs the 2e-2 gate (CPU simulation of the exact pipeline).

Per-core dataflow:
  phase B: per (k, j): AT = att_u8 * cheb_i8 (DVE, bf16 out), then 8
           matmuls RT[(t,f)=128, i=512] += X_j^T @ AT into 8 PSUM
           chains (one per tg/ih), j-accumulated; after each k,
           staggered PSUM->SBUF bf16 casts (DVE even / ACT odd).
           Next k's j0/j1 cheb + products prefetched at j==6.
  phase C: out[i=128, (t,o)] += RT^T @ thetaM_k per tg in order
           0,1(psA) 2,3(psB); thetaM zero-pads Theta[k] per 32-row
           strip so one N=256 matmul covers a whole t-group. relu(psA)
           on ACT, relu(psB) on DVE; final block quarter-split so the
           last store is small. ~14 dummy 512-col matmuls chained on
           the last relu keep PE activity alive so HAM holds the full
           clock through most of the TileContext teardown drains.

DMA schedule (2 HWDGE queues, S=sync A=scalar; receipts, not data
arrival, gate consumers — receipt ~= 9.5us + ~0.9us per 128KB of
queue backlog ahead):
  S: att0h0 | x0 | cheb0h1 | cheb1 | att2 x2 | cheb3 | att4 x4 |
     cheb5 | [chebk1j0] | att6 x6 | cheb7 | ...
  A: cheb0h0 | att0h1 | att1 x1 | cheb2 | att3 x3 | cheb4 |
     att5 x5 | [theta] | [chebk1j1] | cheb6 | att7 x7 | ...
x0 rides 2nd on sync so its receipt (the first-matmul gate) lands
~10.7us instead of 13.1; j0 products come from the slot-1 halves.
WARMUP=31 N=128 matmuls on a zero tile start at PE-queue release
(~7.4us) so the HAM onset clock starts early; they end right as the
first real matmul data lands (~10.8us).
"""

import numpy as np
from contextlib import ExitStack

B, T, N, F_IN, F_OUT, K = 8, 16, 1024, 32, 64, 3
NJ = N // 128  # j tiles (contraction)
NI = N // 128  # i tiles (output rows)
NTG = 4        # t-groups of 4 t's -> 128 = 4*32 partitions
TF = T * F_IN   # 512
TO = T * F_OUT  # 1024
WARMUP = 31
DUMMY_TAIL = 14

_LAST_RESULTS = None  # BassKernelResults of the most recent run (for test harness)


def _build_bass():
    import concourse.mybir as mybir
    import concourse.tile as tile
    from concourse import bacc
    from concourse.bass import ts

    f32 = mybir.dt.float32
    bf16 = mybir.dt.bfloat16
    u8 = mybir.dt.uint8
    i8 = mybir.dt.int8
    nc = bacc.Bacc()

    xT_d = nc.dram_tensor("xT", [N, TF], bf16, kind="ExternalInput")
    attT_d = nc.dram_tensor("attT", [N, N], u8, kind="ExternalInput")
    chebT_d = nc.dram_tensor("chebT", [K * N, N], i8, kind="ExternalInput")
    th_d = nc.dram_tensor("thetaM", [128, K * 4 * F_OUT], bf16, kind="ExternalInput")
    out_d = nc.dram_tensor("out", [N, TO], bf16, kind="ExternalOutput")

    with tile.TileContext(nc) as tc, ExitStack() as ctx:
        x_pool = ctx.enter_context(tc.tile_pool(name="x", bufs=1))
        att_pool = ctx.enter_context(tc.tile_pool(name="att", bufs=1))
        cheb_pool = ctx.enter_context(tc.tile_pool(name="cheb", bufs=5))
        at_pool = ctx.enter_context(tc.tile_pool(name="at", bufs=6))
        rt_pool = ctx.enter_context(tc.tile_pool(name="rt", bufs=K * NTG))
        th_pool = ctx.enter_context(tc.tile_pool(name="th", bufs=1))
        ob_pool = ctx.enter_context(tc.tile_pool(name="ob", bufs=3))
        wz_pool = ctx.enter_context(tc.tile_pool(name="wz", bufs=1))

        q = [nc.sync, nc.scalar]

        xg, attg = [None] * NJ, [None] * NJ

        rts = [[None] * NTG for _ in range(K)]
        th = None
        with tc.tile_pool(name="psumB", bufs=1, space="PSUM") as pb:
            # PE warm-up on a dedicated zero tile: starts the HAM activity
            # clock at PE-queue release; sized to end right as the first
            # real matmul's data receipt lands.
            wz = wz_pool.tile([128, 128], bf16, name="warmz")
            nc.gpsimd.memset(wz[:], 0)
            wps = pb.tile([128, 512], f32, name="warmps", tag="chain7")
            for _ in range(WARMUP):
                nc.tensor.matmul(wps[:, 0:128], wz[:], wz[:], start=True, stop=True)

            at_pre = {}
            for k in range(K):
                chains = [
                    pb.tile([128, 512], f32, name=f"chain{k}_{c}", tag=f"chain{c}")
                    for c in range(2 * NTG)
                ]
                for j in range(NJ):
                    first = k == 0 and j == 0
                    pre = k > 0 and j < 2
                    if k == 0:
                        a = att_pool.tile([128, N], u8,
                                          name=f"att{j}", tag=f"att{j}")
                        xt = x_pool.tile([128, TF], bf16,
                                         name=f"x{j}", tag=f"x{j}")
                        attg[j] = a
                        xg[j] = xt
                    if not pre:
                        cb = cheb_pool.tile([128, N], i8, name=f"cb{k}_{j}",
                                            tag="cb")
                    if k == 0:
                        if first:
                            # slot-1 halves on both queues feed the first
                            # product; x0 whole on sync slot 2 is the
                            # first-matmul gate (receipt ~10.7us)
                            nc.sync.dma_start(a[:, 0:512], attT_d[0:128, 0:512])
                            nc.scalar.dma_start(cb[:, 0:512],
                                                chebT_d[0:128, 0:512])
                            nc.sync.dma_start(xt[:], xT_d[0:128, :])
                            nc.scalar.dma_start(a[:, 512:1024],
                                                attT_d[0:128, 512:1024])
                            nc.sync.dma_start(cb[:, 512:1024],
                                              chebT_d[0:128, 512:1024])
                        else:
                            # att_j+x_j on one queue, cheb_j on the other,
                            # alternating: balanced 384KB per j-pair side
                            qa, qc = q[j % 2], q[(j + 1) % 2]
                            qa.dma_start(a[:], attT_d[ts(j, 128), :])
                            qc.dma_start(cb[:], chebT_d[ts(j, 128), :])
                            qa.dma_start(xt[:], xT_d[ts(j, 128), :])
                    elif not pre:
                        r0 = k * N + j * 128
                        q[(k + j) % 2].dma_start(
                            cb[:], chebT_d[r0:r0 + 128, :])
                    if pre:
                        at = at_pre.pop((k, j))
                    else:
                        at = at_pool.tile([128, N], bf16, name=f"at{k}_{j}",
                                          tag="at")
                        if first:
                            nc.vector.tensor_mul(at[:, 0:512],
                                                 attg[j][:, 0:512],
                                                 cb[:, 0:512])
                            nc.vector.tensor_mul(at[:, 512:1024],
                                                 attg[j][:, 512:1024],
                                                 cb[:, 512:1024])
                        else:
                            nc.vector.tensor_mul(at[:], attg[j][:], cb[:])
                    for ih in range(2):
                        for tg in range(NTG):
                            nc.tensor.matmul(
                                chains[tg * 2 + ih][:],
                                xg[j][:, ts(tg, 128)],
                                at[:, ts(ih, 512)],
                                start=(j == 0),
                                stop=(j == NJ - 1),
                            )
                    if k == 0 and j == 2:
                        # theta only needed by phase C; after the j2 stream
                        th = th_pool.tile([128, K * 4 * F_OUT], bf16)
                        nc.scalar.dma_start(th[:], th_d[:, :])
                    if j == 6 and k < K - 1:
                        # prefetch next k's j0/j1 cheb + products mid-k so
                        # the k-boundary only waits on the chain casts
                        kn = k + 1
                        for jp in range(2):
                            cbp = cheb_pool.tile([128, N], i8,
                                                 name=f"cbp{kn}_{jp}", tag="cb")
                            r0 = kn * N + jp * 128
                            q[jp].dma_start(cbp[:], chebT_d[r0:r0 + 128, :])
                            atp = at_pool.tile([128, N], bf16,
                                               name=f"atp{kn}_{jp}", tag="at")
                            nc.vector.tensor_mul(atp[:], attg[jp][:], cbp[:])
                            at_pre[(kn, jp)] = atp
                for tg in range(NTG):
                    rt = rt_pool.tile([128, N], bf16)
                    nc.vector.tensor_copy(rt[:, 0:512], chains[tg * 2][:])
                    nc.scalar.copy(rt[:, 512:1024], chains[tg * 2 + 1][:])
                    rts[k][tg] = rt

            # phase C: out[i-block, (t,o)] = relu(sum_k RT_k^T @ thetaM_k).
            # One matmul per (tg, k): full K=128 contraction where thetaM
            # zero-pads Theta[k] per 32-row strip, producing the 4 t's of
            # the t-group in one N=256 matmul. tg order 0,1 (psA) then
            # 2,3 (psB): psA is complete at 50% of the i-block so its
            # relu (ACT) and half-block store overlap the psB matmuls;
            # relu(psB) runs on DVE. PSUM tiles reuse the chain{c} tags
            # so each bank frees as soon as its k=2 chain is cast.
            ob = None
            for ib in range(NI):
                psA = pb.tile([128, 512], f32, name=f"psA{ib}",
                              tag=f"chain{(2 * ib) % 8}")
                psB = pb.tile([128, 512], f32, name=f"psB{ib}",
                              tag=f"chain{(2 * ib + 1) % 8}")
                for tg, ps in ((0, psA), (1, psA), (2, psB), (3, psB)):
                    for k in range(K):
                        nc.tensor.matmul(
                            ps[:, ts(tg % 2, 4 * F_OUT)],
                            rts[k][tg][:, ts(ib, 128)],
                            th[:, ts(k, 4 * F_OUT)],
                            start=(k == 0),
                            stop=(k == K - 1),
                        )
                ob = ob_pool.tile([128, TO], bf16)
                nc.scalar.activation(ob[:, 0:512], psA[:],
                                     mybir.ActivationFunctionType.Relu)
                nc.sync.dma_start(out_d[ts(ib, 128), 0:512], ob[:, 0:512])
                if ib < NI - 1:
                    nc.vector.tensor_scalar_max(ob[:, 512:1024], psB[:], 0.0)
                    nc.scalar.dma_start(out_d[ts(ib, 128), 512:1024],
                                        ob[:, 512:1024])
                else:
                    # quarter-split the final block so the very last store is
                    # small and issues right after a short relu
                    nc.vector.tensor_scalar_max(ob[:, 512:768], psB[:, 0:256],
                                                0.0)
                    nc.scalar.dma_start(out_d[ts(ib, 128), 512:768],
                                        ob[:, 512:768])
                    nc.vector.tensor_scalar_max(ob[:, 768:1024], psB[:, 256:512],
                                                0.0)
                    nc.sync.dma_start(out_d[ts(ib, 128), 768:1024],
                                      ob[:, 768:1024])

            # HAM keep-alive: dummy matmuls chained on the final relu (moving
            # operand reads ob) keep PE activity going ~3us into the DMA
            # receipt wait so the teardown drains run at full clock instead
            # of the post-idle half clock. Results are never read.
            dps = pb.tile([128, 512], f32, name="dummyps", tag="chain0")
            for _ in range(DUMMY_TAIL):
                nc.tensor.matmul(dps[:], wz[:], ob[:, 512:1024],
                                 start=True, stop=True)

    nc.compile()
    return nc


def _prep_inputs(x, att, cheb, Theta):
    import ml_dtypes

    bf16 = ml_dtypes.bfloat16
    # per-j int8 scales for cheb; s[j] and the att 1/255 fold into xT rows
    # (exact: RT contracts over j, x carries any per-j factor)
    s = np.abs(cheb).max(axis=(0, 1)) / 127.0          # [N] over (k, i)
    chebT = np.clip(np.rint(cheb / s[None, None, :]), -127, 127).astype(np.int8)
    chebT = np.ascontiguousarray(chebT.transpose(0, 2, 1)).reshape(K * N, N)
    xscale = (s * (1.0 / 255.0)).astype(np.float32)    # [N] per-j factor

    # zero-padded Theta: strip tt of the partition dim carries Theta[k]
    # only in the tt-th 64-col block of k's 256-col group
    thetaM = np.zeros((128, K * 4 * F_OUT), np.float32)
    for tt in range(4):
        for k in range(K):
            thetaM[tt * 32:(tt + 1) * 32,
                   k * 4 * F_OUT + tt * F_OUT:
                   k * 4 * F_OUT + (tt + 1) * F_OUT] = Theta[k]
    thetaM = thetaM.astype(bf16)

    in_maps = []
    for b in range(B):
        xb = np.ascontiguousarray(x[b].transpose(1, 0, 2)).reshape(N, TF)
        in_maps.append({
            "xT": (xb * xscale[:, None]).astype(bf16),
            "attT": np.rint(np.ascontiguousarray(att[b].T) * 255.0
                            ).astype(np.uint8),
            "chebT": chebT,
            "thetaM": thetaM,
        })
    return in_maps


def kernel(**inputs: np.ndarray) -> np.ndarray:
    global _LAST_RESULTS
    from concourse.bass_utils import run_bass_kernel_spmd

    x = np.asarray(inputs["x"], dtype=np.float32)
    att = np.asarray(inputs["spatial_attention"], dtype=np.float32)
    cheb = np.asarray(inputs["cheb"], dtype=np.float32)
    Theta = np.asarray(inputs["Theta"], dtype=np.float32)

    in_maps = _prep_inputs(x, att, cheb, Theta)
    nc = _build_bass()
    res = run_bass_kernel_spmd(nc, in_maps, core_ids=list(range(B)))
    _LAST_RESULTS = res

    out = np.stack(
        [r["out"].astype(np.float32).reshape(N, T, F_OUT).transpose(1, 0, 2)
         for r in res.results]
    )
    return out
